# revision 1
# baseline (speedup 1.0000x reference)
"""Trainium2 Bass kernel for CondorAttention (B=2, S=2048, H=2048, NH=32, HD=64).

Sharding: DP-2 over batch x TP-4 over heads (8 NeuronCores).
Each core computes, for one batch and 8 heads (one 512-wide feature group):
  - QKV projections (bf16 matmuls, fp32 accumulate)
    producing Q^T/K^T in [feature, token] layout and V in [token, feature],
  - RoPE applied in transposed layout (rotate-half via partition-shift DMA),
  - causal attention with *transposed* scores ST[k, q] so the softmax
    denominator comes free from a ones-column appended to V,
  - row-parallel o_proj partial (bf16); the host sums the 4 partials/batch.

The attention_mask input is assumed to be the standard causal mask (checked
on the host; a numpy fallback handles anything else).
"""

import math

import numpy as np

import concourse.bass as bass
import concourse.mybir as mybir
import concourse.tile as tile
from concourse import bacc
from concourse.bass_utils import run_bass_kernel_spmd

F32 = mybir.dt.float32
F32R = mybir.dt.float32r
BF16 = mybir.dt.bfloat16

B, S, H = 2, 2048, 2048
NH, HD = 32, 64
THETA = 10000.0
NCORES = 8
DP, TP = 2, 4
FG = H // TP        # features per core = 512
HPC = NH // TP      # heads per core = 8
NPAIR = HPC // 2    # head pairs = 4
KC = H // 128       # hidden chunks = 16
TT = S // 128       # 128-token tiles = 16
QB = S // 512       # 512-token blocks = 4
NEG = -1.0e9

# emission skew inside the attention inner loop (exp trails S by 1 k-tile,
# PV trails S by 3 k-tiles) so the PE stream never waits on ACT
EXP_DELAY = 1
PV_DELAY = 3


def _emit(tc, xt, wq, wk, wv, wo, cosr, sinr, maskd, y):
    nc = tc.nc

    xt_r = xt.rearrange("(c p) t -> p c t", p=128)      # [128, KC, S]
    wq_r = wq.rearrange("(c p) f -> p c f", p=128)      # [128, KC, FG]
    wk_r = wk.rearrange("(c p) f -> p c f", p=128)
    wv_r = wv.rearrange("(c p) f -> p c f", p=128)
    wo_r = wo.rearrange("(g p) o -> p g o", p=128)      # [128, NPAIR, H]

    from contextlib import ExitStack

    with ExitStack() as ctx:
        consts = ctx.enter_context(tc.tile_pool(name="consts", bufs=1))
        rope_pool = ctx.enter_context(tc.tile_pool(name="ropep", bufs=2))
        qk_pool = ctx.enter_context(tc.tile_pool(name="qkp", bufs=4))
        v1_pool = ctx.enter_context(tc.tile_pool(name="v1p", bufs=1))
        pt_pool = ctx.enter_context(tc.tile_pool(name="ptp", bufs=8))
        nrm_pool = ctx.enter_context(tc.tile_pool(name="nrmp", bufs=2))
        o_pool = ctx.enter_context(tc.tile_pool(name="op", bufs=4))

        mm_pool = ctx.enter_context(tc.tile_pool(name="mmps", bufs=2,
                                                 space="PSUM"))
        st_pool = ctx.enter_context(tc.tile_pool(name="stps", bufs=4,
                                                 space="PSUM"))
        ot_pool = ctx.enter_context(tc.tile_pool(name="otps", bufs=2,
                                                 space="PSUM"))

        # Input pools live only through the projections; their SBUF is
        # reclaimed for Wo/Y in stage C.
        in_ctx = ExitStack()
        xt_pool = in_ctx.enter_context(tc.tile_pool(name="xtp", bufs=1))
        wv_pool = in_ctx.enter_context(tc.tile_pool(name="wvp", bufs=1))
        wqk_pool = in_ctx.enter_context(tc.tile_pool(name="wqkp", bufs=2))

        # ---- load Wv then X^T first (V-proj matmuls gate on these) ----
        wv_sb = wv_pool.tile([128, KC, FG], BF16)
        nc.sync.dma_start(out=wv_sb, in_=wv_r)
        xt_sb = xt_pool.tile([128, KC, S], BF16)
        for ti in range(TT):
            nc.sync.dma_start(
                out=xt_sb[:, :, ti * 128:(ti + 1) * 128],
                in_=xt_r[:, :, ti * 128:(ti + 1) * 128],
            )

        # ---- constants (needed later, after the first projections) ----
        cos_sb = consts.tile([128, S], F32)
        sin_sb = consts.tile([128, S], F32)
        mask_sb = consts.tile([128, 128], F32)
        ones_sb = consts.tile([65, 64], F32)
        nc.sync.dma_start(out=cos_sb, in_=cosr)
        nc.sync.dma_start(out=sin_sb, in_=sinr)
        nc.sync.dma_start(out=mask_sb, in_=maskd)
        nc.vector.memset(ones_sb, 1.0)

        # ---- V projection -> V1 (bf16 per head, 64 ones cols fused) ---
        # Even heads use lhsT = [V | ones]: PV rows 0-63, softmax denom
        # replicated on rows 64-127. Odd heads use [ones | V] so their
        # normalized output lands directly on SBUF partitions 64-127.
        v1_sb = v1_pool.tile([128, TT, HPC, 128], BF16)
        nc.vector.memset(v1_sb[:, :, 0::2, 64:128], 1.0)
        nc.vector.memset(v1_sb[:, :, 1::2, 0:64], 1.0)

        for ti in range(TT):
            vps = mm_pool.tile([128, FG], F32, tag="mm")
            for hc in range(KC):
                nc.tensor.matmul(
                    vps,
                    xt_sb[:, hc, ti * 128:(ti + 1) * 128],
                    wv_sb[:, hc, :],
                    start=(hc == 0),
                    stop=(hc == KC - 1),
                )
            vps_h = vps.rearrange("p (h d) -> p h d", h=HPC)
            nc.vector.tensor_copy(
                out=v1_sb[:, ti, 0::2, 0:64], in_=vps_h[:, 0::2, :])
            nc.vector.tensor_copy(
                out=v1_sb[:, ti, 1::2, 64:128], in_=vps_h[:, 1::2, :])

        # ---- RoPE helper ----------------------------------------------
        def rope(psrc, tb, dst):
            ts_ = slice(tb * 512, (tb + 1) * 512)
            raw = rope_pool.tile([128, 512], F32, tag="raw")
            shf = rope_pool.tile([128, 512], F32, tag="shf")
            nc.vector.tensor_copy(out=raw, in_=psrc)
            # rotate-half: swap 32-blocks within each 64-block
            nc.sync.dma_start(out=shf[0:32, :], in_=raw[32:64, :])
            nc.sync.dma_start(out=shf[32:64, :], in_=raw[0:32, :])
            nc.sync.dma_start(out=shf[64:96, :], in_=raw[96:128, :])
            nc.sync.dma_start(out=shf[96:128, :], in_=raw[64:96, :])
            nc.vector.tensor_mul(raw, raw, cos_sb[:, ts_])
            nc.vector.tensor_mul(shf, shf, sin_sb[:, ts_])
            nc.vector.tensor_add(dst[:, ts_], raw, shf)

        # ---- projection sub-units (4 matmuls each) for interleaving ---
        def start_pair_proj(hp):
            fs = slice(hp * 128, (hp + 1) * 128)
            wq_sb = wqk_pool.tile([128, KC, 128], BF16, tag="wqk")
            wk_sb = wqk_pool.tile([128, KC, 128], BF16, tag="wqk")
            nc.sync.dma_start(out=wq_sb, in_=wq_r[:, :, fs])
            nc.sync.dma_start(out=wk_sb, in_=wk_r[:, :, fs])
            qt_sb = qk_pool.tile([128, S], BF16, tag="qk")
            kt_sb = qk_pool.tile([128, S], BF16, tag="qk")

            units = []
            for w_sb, dst in ((wq_sb, qt_sb), (wk_sb, kt_sb)):
                for tb in range(QB):
                    pps_box = []

                    def mm4(h0, w_sb=w_sb, tb=tb, pps_box=pps_box):
                        if h0 == 0:
                            pps_t = mm_pool.tile([128, 512], F32, tag="mm")
                            pps_box.append(pps_t)
                        pps = pps_box[0]
                        for hc in range(h0, h0 + 4):
                            nc.tensor.matmul(
                                pps,
                                w_sb[:, hc, :],
                                xt_sb[:, hc, tb * 512:(tb + 1) * 512],
                                start=(hc == 0),
                                stop=(hc == KC - 1),
                            )
                    for h0 in range(0, KC, 4):
                        units.append(lambda h0=h0, mm4=mm4: mm4(h0))
                    units.append(lambda tb=tb, dst=dst, pps_box=pps_box:
                                 rope(pps_box[0], tb, dst))
            return qt_sb, kt_sb, units

        # ---- stage C units (one PSUM Y tile each) ---------------------
        wo_box = []
        y_parts = []

        def c_unit(ti, oc):
            wo_sb = wo_box[0]
            t_sl = slice(ti * 128, (ti + 1) * 128)
            o_sl = slice(oc * 512, (oc + 1) * 512)
            yps = mm_pool.tile([128, 512], F32, tag="mm")
            for g in range(NPAIR):
                nc.tensor.matmul(
                    yps,
                    o_tiles[g][:, t_sl],
                    wo_sb[:, g, o_sl],
                    start=(g == 0),
                    stop=(g == NPAIR - 1),
                )
            ysb = y_pool.tile([128, 512], F32, tag="y")
            nc.scalar.copy(ysb, yps)
            nc.sync.dma_start(out=y[t_sl, o_sl], in_=ysb)

        # ---- normalization helpers ------------------------------------
        # dcol row 64 holds the even head's denominators, row 0 the odd
        # head's. 1/denom = exp(-ln(denom)) in place on ScalarE; the
        # reciprocal row is broadcast across 64 partitions with a rank-1
        # fp32r matmul (ones column), landing in PSUM for the final mul.
        def recip_rows(dcol, cols):
            for r in (0, 64):
                nc.scalar.activation(dcol[r:r + 1, cols], dcol[r:r + 1, cols],
                                     mybir.ActivationFunctionType.Ln)
            for r in (0, 64):
                nc.scalar.activation(dcol[r:r + 1, cols], dcol[r:r + 1, cols],
                                     mybir.ActivationFunctionType.Exp,
                                     scale=-1.0)

        def apply_norm(o_sb, dcol, qj):
            o_sl = slice(qj * 512, (qj + 1) * 512)
            r2p = mm_pool.tile([128, 512], F32, tag="mm")
            nc.tensor.matmul(
                r2p[0:64, :], ones_sb[64:65, :],
                dcol[64:65, o_sl], tile_position=(64, 0),
            )
            nc.tensor.matmul(
                r2p[64:128, :], ones_sb[0:1, :],
                dcol[0:1, o_sl], tile_position=(0, 64),
            )
            nc.vector.tensor_mul(o_sb[0:64, o_sl], o_sb[0:64, o_sl],
                                 r2p[0:64, :])
            nc.vector.tensor_mul(o_sb[64:128, o_sl], o_sb[64:128, o_sl],
                                 r2p[64:128, :])

        o_tiles = []

        # pair 0's projections run up front; later pairs' projections and
        # stage C interleave into attention as PE filler work
        qt_kt = [None] * NPAIR
        qt_kt[0] = start_pair_proj(0)
        for u in qt_kt[0][2]:
            u()

        for hp in range(NPAIR):
            qt_sb, kt_sb, _ = qt_kt[hp]
            filler = []
            if hp + 1 < NPAIR:
                qt_kt[hp + 1] = start_pair_proj(hp + 1)
                filler = list(qt_kt[hp + 1][2])
            if hp == NPAIR - 1:
                # xt/wv/wqk done (pair-3 proj ran during pair-2); reclaim
                # their SBUF for Wo and the Y staging tiles
                in_ctx.close()
                wo_pool = ctx.enter_context(tc.tile_pool(name="wop", bufs=1))
                y_pool = ctx.enter_context(tc.tile_pool(name="yp", bufs=2))
                wo_sb = wo_pool.tile([128, NPAIR, H], BF16)
                nc.sync.dma_start(out=wo_sb, in_=wo_r)
                wo_box.append(wo_sb)

            o_sb = o_pool.tile([128, S], BF16, tag="o")
            o_tiles.append(o_sb)
            dcol = nrm_pool.tile([65, S], F32, tag="dcol")

            for qj in range(QB):
                nki = 4 * qj + 4
                ot_a = ot_pool.tile([128, 512], F32, tag="ot")
                ot_b = ot_pool.tile([128, 512], F32, tag="ot")
                sts = {}
                pts = {}

                def s_step(ki, qj=qj, sts=sts, qt_sb=qt_sb, kt_sb=kt_sb):
                    q0 = max(128 * ki, 512 * qj)
                    n = 512 * (qj + 1) - q0
                    qs = slice(q0, q0 + n)
                    ks = slice(128 * ki, 128 * (ki + 1))
                    sta = st_pool.tile([128, 512], F32, tag="st")
                    stb = st_pool.tile([128, 512], F32, tag="st")
                    nc.tensor.matmul(
                        sta[:, 0:n], kt_sb[0:64, ks], qt_sb[0:64, qs],
                        tile_position=(0, 0),
                    )
                    nc.tensor.matmul(
                        stb[:, 0:n], kt_sb[64:128, ks], qt_sb[64:128, qs],
                        tile_position=(64, 0),
                    )
                    if 128 * ki >= 512 * qj:  # diagonal tile: causal mask
                        nc.vector.tensor_add(sta[:, 0:128], sta[:, 0:128],
                                             mask_sb)
                        nc.vector.tensor_add(stb[:, 0:128], stb[:, 0:128],
                                             mask_sb)
                    sts[ki] = (sta, stb, n)

                def e_step(ki, sts=sts, pts=pts):
                    sta, stb, n = sts.pop(ki)
                    pta = pt_pool.tile([128, 512], BF16, tag="pt")
                    ptb = pt_pool.tile([128, 512], BF16, tag="pt")
                    nc.scalar.activation(
                        pta[:, 0:n], sta[:, 0:n],
                        mybir.ActivationFunctionType.Exp,
                        scale=1.0 / math.sqrt(HD),
                    )
                    nc.scalar.activation(
                        ptb[:, 0:n], stb[:, 0:n],
                        mybir.ActivationFunctionType.Exp,
                        scale=1.0 / math.sqrt(HD),
                    )
                    pts[ki] = (pta, ptb, n)

                def pv_step(ki, nki=nki, hp=hp, ot_a=ot_a, ot_b=ot_b,
                            pts=pts):
                    pta, ptb, n = pts.pop(ki)
                    c0 = 512 - n
                    nc.tensor.matmul(
                        ot_a[:, c0:512], v1_sb[:, ki, 2 * hp, :], pta[:, 0:n],
                        start=(ki == 0), stop=(ki == nki - 1),
                        skip_group_check=True,
                    )
                    nc.tensor.matmul(
                        ot_b[:, c0:512], v1_sb[:, ki, 2 * hp + 1, :],
                        ptb[:, 0:n],
                        start=(ki == 0), stop=(ki == nki - 1),
                        skip_group_check=True,
                    )

                for ki in range(nki + PV_DELAY):
                    if ki < nki:
                        s_step(ki)
                    if EXP_DELAY <= ki and ki - EXP_DELAY < nki:
                        e_step(ki - EXP_DELAY)
                    if PV_DELAY <= ki and ki - PV_DELAY < nki:
                        pv_step(ki - PV_DELAY)
                    if filler:
                        filler.pop(0)()

                # stash unnormalized PV rows + the denominator rows
                o_sl = slice(qj * 512, (qj + 1) * 512)
                for head_ot, odd in ((ot_a, False), (ot_b, True)):
                    dnr = 0 if odd else 64
                    pv = slice(64, 128) if odd else slice(0, 64)
                    nc.vector.tensor_copy(out=o_sb[pv, o_sl],
                                          in_=head_ot[pv, :])
                    nc.vector.tensor_copy(out=dcol[dnr:dnr + 1, o_sl],
                                          in_=head_ot[dnr:dnr + 1, :])

                if hp == NPAIR - 1:
                    # last pair: normalize per q-block so stage C can
                    # start on its token tiles right away
                    recip_rows(dcol, o_sl)
                    apply_norm(o_sb, dcol, qj)
                    for ti in range(4 * qj, 4 * qj + 4):
                        for oc in range(QB):
                            filler.append(
                                lambda ti=ti, oc=oc: c_unit(ti, oc))
                elif filler:
                    filler.pop(0)()
                    filler.pop(0)()

            if hp < NPAIR - 1:
                # batched pair-end normalization (2 ACT table swaps/pair)
                recip_rows(dcol, slice(0, S))
                for qj in range(QB):
                    apply_norm(o_sb, dcol, qj)
            else:
                for u in filler:
                    u()


def build_program():
    nc = bacc.Bacc("TRN2", target_bir_lowering=False, debug=False,
                   num_devices=NCORES)
    xt = nc.dram_tensor("xt", [H, S], BF16, kind="ExternalInput").ap()
    wq = nc.dram_tensor("wq", [H, FG], BF16, kind="ExternalInput").ap()
    wk = nc.dram_tensor("wk", [H, FG], BF16, kind="ExternalInput").ap()
    wv = nc.dram_tensor("wv", [H, FG], BF16, kind="ExternalInput").ap()
    wo = nc.dram_tensor("wo", [FG, H], BF16, kind="ExternalInput").ap()
    cosr = nc.dram_tensor("cosr", [128, S], F32, kind="ExternalInput").ap()
    sinr = nc.dram_tensor("sinr", [128, S], F32, kind="ExternalInput").ap()
    maskd = nc.dram_tensor("maskd", [128, 128], F32, kind="ExternalInput").ap()
    y = nc.dram_tensor("y", [S, H], F32, kind="ExternalOutput").ap()

    with tile.TileContext(nc) as tc:
        _emit(tc, xt, wq, wk, wv, wo, cosr, sinr, maskd, y)
    nc.compile()
    return nc


def host_tables():
    """cos/sin tables in transposed, pair-replicated, sign-folded layout."""
    inv_freq = 1.0 / (THETA ** (np.arange(0, HD, 2, dtype=np.float32) / HD))
    t = np.arange(S, dtype=np.float32)
    freqs = np.einsum("t,f->tf", t, inv_freq)          # [S, 32]
    cos64 = np.cos(np.concatenate([freqs, freqs], 1))  # [S, 64]
    sin64 = np.sin(np.concatenate([freqs, freqs], 1))
    # sign-folded sin: rows (d<32) multiply the shifted *upper* half by -sin
    sgn = np.where(np.arange(HD) < 32, -1.0, 1.0).astype(np.float32)
    sin64 = sin64 * sgn[None, :]
    cosr = np.ascontiguousarray(np.tile(cos64.T, (2, 1)))  # [128, S]
    sinr = np.ascontiguousarray(np.tile(sin64.T, (2, 1)))
    return cosr, sinr


def host_mask():
    """Additive causal mask for the diagonal ST tile [k_local, q_local]:
    NEG where q_local < k_local."""
    kl = np.arange(128)[:, None]
    ql = np.arange(128)[None, :]
    return np.where(ql < kl, NEG, 0.0).astype(np.float32)


def _expected_causal_mask(attention_mask):
    causal = np.tril(np.ones((S, S), dtype=bool))
    want = np.where(causal, 0.0, NEG).astype(np.float32)
    return np.array_equal(np.asarray(attention_mask).reshape(S, S), want)


def _numpy_fallback(hidden_states, attention_mask, wq, wk, wv, wo):
    x = np.asarray(hidden_states, dtype=np.float32)
    b, s, _ = x.shape
    q = (x @ wq).reshape(b, s, NH, HD).transpose(0, 2, 1, 3)
    k = (x @ wk).reshape(b, s, NH, HD).transpose(0, 2, 1, 3)
    v = (x @ wv).reshape(b, s, NH, HD).transpose(0, 2, 1, 3)
    inv_freq = 1.0 / (THETA ** (np.arange(0, HD, 2, dtype=np.float32) / HD))
    t = np.arange(s, dtype=np.float32)
    freqs = np.einsum("t,f->tf", t, inv_freq)
    emb = np.concatenate([freqs, freqs], axis=-1)
    cos, sin = np.cos(emb)[None, None], np.sin(emb)[None, None]

    def rot(z):
        z1, z2 = z[..., :HD // 2], z[..., HD // 2:]
        return np.concatenate([-z2, z1], axis=-1)

    q = q * cos + rot(q) * sin
    k = k * cos + rot(k) * sin
    attn = np.einsum("bhqd,bhkd->bhqk", q, k) / math.sqrt(HD)
    attn = attn + np.asarray(attention_mask)
    attn = attn - attn.max(axis=-1, keepdims=True)
    w = np.exp(attn)
    w = w / w.sum(axis=-1, keepdims=True)
    out = np.einsum("bhqk,bhkd->bhqd", w, v)
    out = out.transpose(0, 2, 1, 3).reshape(b, s, H)
    return (out @ wo).astype(np.float32)


_NC_CACHE = {}
_RUN_KWARGS = {}


def kernel(hidden_states, attention_mask, wq, wk, wv, wo):
    import ml_dtypes

    hidden_states = np.asarray(hidden_states, dtype=np.float32)
    attention_mask = np.asarray(attention_mask, dtype=np.float32)
    wq = np.asarray(wq, dtype=np.float32)
    wk = np.asarray(wk, dtype=np.float32)
    wv = np.asarray(wv, dtype=np.float32)
    wo = np.asarray(wo, dtype=np.float32)

    if hidden_states.shape != (B, S, H) or not _expected_causal_mask(
            attention_mask):
        return _numpy_fallback(hidden_states, attention_mask, wq, wk, wv, wo)

    if "nc" not in _NC_CACHE:
        _NC_CACHE["nc"] = build_program()
    nc = _NC_CACHE["nc"]

    cosr, sinr = host_tables()
    maskd = host_mask()

    in_maps = []
    for core in range(NCORES):
        b = core // TP
        g = core % TP
        fsl = slice(g * FG, (g + 1) * FG)
        in_maps.append({
            "xt": np.ascontiguousarray(hidden_states[b].T).astype(ml_dtypes.bfloat16),
            "wq": np.ascontiguousarray(wq[:, fsl]).astype(ml_dtypes.bfloat16),
            "wk": np.ascontiguousarray(wk[:, fsl]).astype(ml_dtypes.bfloat16),
            "wv": np.ascontiguousarray(wv[:, fsl]).astype(ml_dtypes.bfloat16),
            "wo": np.ascontiguousarray(wo[fsl, :]).astype(ml_dtypes.bfloat16),
            "cosr": cosr,
            "sinr": sinr,
            "maskd": maskd,
        })

    res = run_bass_kernel_spmd(nc, in_maps, core_ids=list(range(NCORES)),
                               **_RUN_KWARGS)
    _NC_CACHE["last_results"] = res
    out = np.zeros((B, S, H), dtype=np.float32)
    for core in range(NCORES):
        out[core // TP] += res.results[core]["y"]
    return out

